# revision 1
# baseline (speedup 1.0000x reference)
"""Trainium2 Bass kernel for causal self-attention with RoPE.

Problem: x[2,2048,2048] f32, w_qkv[6144,2048], w_out[2048,2048].
  qkv = x @ w_qkv.T ; split into 16 heads of 128; RoPE on q,k;
  causal softmax attention; out = attn_out @ w_out.T.

Sharding (8 cores): core c -> batch b = c//4, head-group g = c%4
(4 heads each). Each core computes a partial output projection for its
heads; the host sums the 4 partials per batch.

Device-side layout strategy (per core):
  - Host feeds x^T (xt: [D,t]) and pre-transposed weight shards so every
    matmul operand has its contraction dim on SBUF partitions natively.
  - Phase A: qk^T tiles [j,t] = wqk^T.T @ x^T, with RoPE fused at PSUM
    eviction: q_rot = q*cosT + (S@q)*sinT2, where S (sign-swap for
    rotate_half) is applied with a 128x128 PE matmul.
  - Phase B: v in natural layout [t,j] = x^T.T @ wv^T, augmented with a
    ones column (index 128) per head.
  - Attention per head: scoresT[tk,tq] = krot.T @ qrot (PSUM), additive
    causal mask on diagonal subtiles, Exp on ACT (scale=1/sqrt(128)) ->
    SBUF, then AV matmuls lhsT=exp-tile rhs=v_aug accumulate
    [tq, 0:129] in PSUM; column 128 is the softmax denominator.
    Normalize with per-partition reciprocal, PE-transpose Y -> Y^T.
  - Projection: out[t,o] += Y_h^T.T @ w_out^T rows of head h.

All matmuls run as float32r (full-rate fp32 on the PE), accumulating in
fp32 PSUM.
"""

import os
import sys
import time
from contextlib import ExitStack

import numpy as np

if "/opt/trn_rl_repo" not in sys.path:
    sys.path.insert(0, "/opt/trn_rl_repo")

import concourse.bass as bass  # noqa: E402
import concourse.mybir as mybir  # noqa: E402
import concourse.tile as tile  # noqa: E402
from concourse import bacc  # noqa: E402
from concourse import bass_utils  # noqa: E402
from concourse.masks import make_identity  # noqa: E402

P = 128
T = 2048
DIM = 2048
HD = 128
NH = 4  # heads per core
TGQ = 256  # t-group width for the qk projection phase
QG = 512  # tq group width in attention
SCALE = float(HD) ** -0.5
F32 = mybir.dt.float32
F32R = mybir.dt.float32r
VBW = HD + 4  # v block width per (t-tile, head): 128 data + 1 ones + 3 pad
AVN = 256  # AV matmul moving free dim (>=256 keeps fp32r at full rate)
VPAD = AVN - VBW  # tail padding so the last block can read AVN columns

_CACHE: dict = {}
LAST_RESULTS = None


def _build_program(t=T, dim=DIM):
    ndt = dim // P  # contraction tiles over D
    ntt = t // P  # token tiles
    ntg = t // TGQ
    nqg = t // QG
    nog = dim // 512

    nc = bacc.Bacc("TRN2", target_bir_lowering=False, debug=False)

    xt_d = nc.dram_tensor("xt", [dim, t], F32, kind="ExternalInput")
    wqk_d = nc.dram_tensor("wqkt", [dim, 2 * NH * HD], F32, kind="ExternalInput")
    wv_d = nc.dram_tensor("wvt", [dim, NH * HD], F32, kind="ExternalInput")
    wo_d = nc.dram_tensor("wot", [NH * HD, dim], F32, kind="ExternalInput")
    cos_d = nc.dram_tensor("cost", [P, t], F32, kind="ExternalInput")
    sin_d = nc.dram_tensor("sint", [P, t], F32, kind="ExternalInput")
    st_d = nc.dram_tensor("st", [P, P], F32, kind="ExternalInput")
    mask_d = nc.dram_tensor("mask", [P, P], F32, kind="ExternalInput")
    vones_d = nc.dram_tensor("vones", [P, ntt * NH, 4], F32, kind="ExternalInput")
    vtail_d = nc.dram_tensor("vtail", [P, VPAD], F32, kind="ExternalInput")
    out_d = nc.dram_tensor("out", [t, dim], F32, kind="ExternalOutput")

    with ExitStack() as ctx:
        tc = ctx.enter_context(tile.TileContext(nc))

        const = ctx.enter_context(tc.tile_pool(name="const", bufs=1))
        st_sb = const.tile([P, P], F32R)
        mask_sb = const.tile([P, P], F32)
        ident = const.tile([P, P], F32)
        nc.sync.dma_start(st_sb[:], st_d.ap().bitcast(F32R))
        nc.sync.dma_start(mask_sb[:], mask_d.ap())
        make_identity(nc, ident[:])

        # persistent rotated q/k
        qk_pool = ctx.enter_context(tc.tile_pool(name="qk", bufs=1))
        qk_sb = qk_pool.tile([P, 2 * NH, t], F32R)  # rotated qT (0:4) / kT (4:8)

        # wv outlives phase A (prefetched during it, consumed in phase B)
        wv_pool = ctx.enter_context(tc.tile_pool(name="wv", bufs=1))

        # ---------------- Phase A: q/k projection + RoPE ----------------
        with (
            tc.tile_pool(name="wqk", bufs=1) as wqk_pool,
            tc.tile_pool(name="xta", bufs=2) as xt_pool,
            tc.tile_pool(name="trig", bufs=2) as trig_pool,
            tc.tile_pool(name="ropes", bufs=2) as rope_pool,
            tc.tile_pool(name="psqk", bufs=6, space="PSUM") as ps_qk_pool,
            tc.tile_pool(name="psrope", bufs=2, space="PSUM") as ps_rope_pool,
        ):
            # cold-start critical path: interleave weight-d and xt(tg0)-d
            wqk_sb = []
            xt0 = []
            for d in range(ndt):
                w_t = wqk_pool.tile(
                    [P, 2 * NH * HD], F32R, name=f"wqk_{d}", tag=f"wqk{d}"
                )
                eng = nc.sync if d % 2 == 0 else nc.scalar
                eng.dma_start(
                    w_t[:], wqk_d.ap()[d * P : (d + 1) * P, :].bitcast(F32R)
                )
                wqk_sb.append(w_t)
                x_t = xt_pool.tile([P, TGQ], F32R, name=f"xta_0_{d}", tag=f"xta{d}")
                nc.sync.dma_start(
                    x_t[:], xt_d.ap()[d * P : (d + 1) * P, 0:TGQ].bitcast(F32R)
                )
                xt0.append(x_t)
            cos0 = trig_pool.tile([P, TGQ], F32, name="cos_0", tag="cos")
            sin0 = trig_pool.tile([P, TGQ], F32, name="sin_0", tag="sin")
            nc.sync.dma_start(cos0[:], cos_d.ap()[:, 0:TGQ])
            nc.sync.dma_start(sin0[:], sin_d.ap()[:, 0:TGQ])
            wv_sb = []

            for tg in range(ntg):
                t0 = tg * TGQ
                if tg == 0:
                    xt_t, cos_t, sin_t = xt0, cos0, sin0
                else:
                    xt_t = [
                        xt_pool.tile(
                            [P, TGQ], F32R, name=f"xta_{tg}_{d}", tag=f"xta{d}"
                        )
                        for d in range(ndt)
                    ]
                    for d in range(ndt):
                        nc.sync.dma_start(
                            xt_t[d][:],
                            xt_d.ap()[d * P : (d + 1) * P, t0 : t0 + TGQ].bitcast(
                                F32R
                            ),
                        )
                    cos_t = trig_pool.tile([P, TGQ], F32, name=f"cos_{tg}", tag="cos")
                    sin_t = trig_pool.tile([P, TGQ], F32, name=f"sin_{tg}", tag="sin")
                    nc.sync.dma_start(cos_t[:], cos_d.ap()[:, t0 : t0 + TGQ])
                    nc.sync.dma_start(sin_t[:], sin_d.ap()[:, t0 : t0 + TGQ])
                if tg == 1:
                    # prefetch phase-B weights once the phase-A stream is warm
                    for d in range(ndt):
                        wv_t = wv_pool.tile(
                            [P, NH * HD], F32R, name=f"wv_{d}", tag=f"wv{d}"
                        )
                        nc.sync.dma_start(
                            wv_t[:], wv_d.ap()[d * P : (d + 1) * P, :].bitcast(F32R)
                        )
                        wv_sb.append(wv_t)
                for jt in range(2 * NH):
                    ps = ps_qk_pool.tile([P, TGQ], F32)
                    for d in range(ndt):
                        nc.tensor.matmul(
                            ps[:],
                            wqk_sb[d][:, jt * P : (jt + 1) * P],
                            xt_t[d][:],
                            start=(d == 0),
                            stop=(d == ndt - 1),
                        )
                    # RoPE: rot = raw*cosT + (S@raw)*sinT2
                    raw = rope_pool.tile([P, TGQ], F32R, tag="raw")
                    nc.scalar.copy(raw[:], ps[:])
                    ps2 = ps_rope_pool.tile([P, TGQ], F32)
                    nc.tensor.matmul(ps2[:], st_sb[:], raw[:], start=True, stop=True)
                    t1 = rope_pool.tile([P, TGQ], F32, tag="t1")
                    nc.vector.tensor_mul(t1[:], ps2[:], sin_t[:])
                    t2 = rope_pool.tile([P, TGQ], F32, tag="t2")
                    nc.gpsimd.tensor_mul(t2[:], raw[:].bitcast(F32), cos_t[:])
                    nc.vector.tensor_add(
                        qk_sb[:, jt, t0 : t0 + TGQ], t2[:], t1[:]
                    )

        v_pool = ctx.enter_context(tc.tile_pool(name="vb", bufs=1))
        # flat v: blocks of VBW per (t-tile, head) + VPAD zero tail so AV
        # matmuls can read AVN contiguous columns from any block start
        v_sb = v_pool.tile([P, ntt * NH * VBW + VPAD], F32R)

        # PSUM pools for phases B..D share one scope: B's accumulators ride
        # the "psy" slots, leaving the score banks free so phase C's
        # score/exp pipeline (needs only q/k) can overlap phase B.
        ps_s_pool = ctx.enter_context(tc.tile_pool(name="pss", bufs=3, space="PSUM"))
        ps_y_pool = ctx.enter_context(tc.tile_pool(name="psy", bufs=4, space="PSUM"))
        ps_o_pool = ctx.enter_context(tc.tile_pool(name="pso", bufs=1, space="PSUM"))

        # ---------------- Phase B: v projection ----------------
        with (
            tc.tile_pool(name="xtb", bufs=3) as xtb_pool,
        ):
            # ones column (idx 128) + zero padding (129..131) per head block
            nc.sync.dma_start(
                v_sb[:, 0 : ntt * NH * VBW].rearrange("p (a c) -> p a c", c=VBW)[
                    :, :, HD : HD + 4
                ],
                vones_d.ap().bitcast(F32R),
            )
            nc.sync.dma_start(
                v_sb[:, ntt * NH * VBW :], vtail_d.ap().bitcast(F32R)
            )
            for tg in range(ntg):
                t0 = tg * TGQ
                xt_t = [
                    xtb_pool.tile([P, TGQ], F32R, name=f"xtb_{tg}_{d}", tag=f"xtb{d}")
                    for d in range(ndt)
                ]
                for d in range(ndt):
                    nc.sync.dma_start(
                        xt_t[d][:],
                        xt_d.ap()[d * P : (d + 1) * P, t0 : t0 + TGQ].bitcast(F32R),
                    )
                for sb in range(TGQ // P):
                    tt = (t0 // P) + sb
                    ps = ps_y_pool.tile(
                        [P, NH * HD], F32, name=f"psv_{tt}", tag="psy"
                    )
                    for d in range(ndt):
                        nc.tensor.matmul(
                            ps[:],
                            xt_t[d][:, sb * P : (sb + 1) * P],
                            wv_sb[d][:],
                            start=(d == 0),
                            stop=(d == ndt - 1),
                        )
                    for h in range(NH):
                        off = (tt * NH + h) * VBW
                        nc.vector.tensor_copy(
                            v_sb[:, off : off + HD], ps[:, h * HD : (h + 1) * HD]
                        )

        # normalized attn out Y^T, split per (head, q-group) for fine deps
        yt_pool = ctx.enter_context(tc.tile_pool(name="yt", bufs=1))
        yt_sb = [
            [
                yt_pool.tile([P, QG], F32R, name=f"yt_{h}_{G}", tag=f"yt{h}_{G}")
                for G in range(nqg)
            ]
            for h in range(NH)
        ]

        # ---------------- Phases C+D: attention + projection ----------------
        # PSUM: pss 4 banks + shared pool (psy accumulators, Y transposes,
        # projection) 4 banks. Projection for q-group G is emitted right
        # after (G, h=3), so its matmuls fill ACT-bound attention bubbles.
        with (
            tc.tile_pool(name="expt", bufs=3) as exp_pool,
            tc.tile_pool(name="ynorm", bufs=4) as y_pool,
            tc.tile_pool(name="recip", bufs=4) as r_pool,
            tc.tile_pool(name="wo", bufs=1) as wo_pool,
            tc.tile_pool(name="ob", bufs=2) as out_pool,
        ):
            wo_sb = []
            for og in range(nog):
                wo_t = wo_pool.tile(
                    [P, NH, 512], F32R, name=f"wo_{og}", tag=f"wo{og}"
                )
                for h in range(NH):
                    nc.sync.dma_start(
                        wo_t[:, h, :],
                        wo_d.ap()[
                            h * P : (h + 1) * P, og * 512 : (og + 1) * 512
                        ].bitcast(F32R),
                    )
                wo_sb.append(wo_t)

            def emit_proj(Gp, tt, og, pool_pick=0):
                # projection of token tile tt (q-group Gp), one o-group
                if pool_pick == 1:
                    ps = ps_s_pool.tile(
                        [P, QG], F32, name=f"pso_{og}_{tt}", tag="ps_s"
                    )
                elif pool_pick == 2:
                    ps = ps_y_pool.tile(
                        [P, 2 * AVN], F32, name=f"pso_{og}_{tt}", tag="psy"
                    )
                else:
                    ps = ps_o_pool.tile(
                        [P, 2 * AVN], F32, name=f"pso_{og}_{tt}", tag="pso"
                    )
                for h in range(NH):
                    nc.tensor.matmul(
                        ps[:, 0:512],
                        yt_sb[h][Gp][:, (tt % 4) * P : (tt % 4 + 1) * P],
                        wo_sb[og][:, h, :],
                        start=(h == 0),
                        stop=(h == NH - 1),
                    )
                ob = out_pool.tile([P, 512], F32, tag="ob")
                if (og + tt) % 2 == 0:
                    nc.scalar.copy(ob[:], ps[:, 0:512])
                else:
                    nc.vector.tensor_copy(ob[:], ps[:, 0:512])
                nc.sync.dma_start(
                    out_d.ap()[tt * P : (tt + 1) * P, og * 512 : (og + 1) * 512],
                    ob[:],
                )

            for G in range(nqg):
                for h in range(NH):
                    qrot = qk_sb[:, h, :]
                    krot = qk_sb[:, NH + h, :]
                    q0 = G * QG
                    ps_y = [
                        ps_y_pool.tile(
                            [P, 2 * AVN], F32, name=f"psy_{h}_{G}_{gi}", tag="psy"
                        )
                        for gi in range(4)
                    ]
                    for j in range(4 * G + 4):
                        ps_s = ps_s_pool.tile([P, QG], F32)
                        nc.tensor.matmul(
                            ps_s[:],
                            krot[:, j * P : (j + 1) * P],
                            qrot[:, q0 : q0 + QG],
                            start=True,
                            stop=True,
                        )
                        k0 = 0  # first live q-subtile for this tk tile
                        if j >= 4 * G:
                            k0 = j - 4 * G
                            sl = slice(k0 * P, (k0 + 1) * P)
                            nc.vector.tensor_add(ps_s[:, sl], ps_s[:, sl], mask_sb[:])
                        ex = exp_pool.tile([P, QG], F32R)
                        nc.scalar.activation(
                            ex[:, k0 * P : QG], ps_s[:, k0 * P : QG],
                            mybir.ActivationFunctionType.Exp,
                            scale=SCALE,
                        )
                        for gi in range(4):
                            i = 4 * G + gi
                            if j <= i:
                                voff = (j * NH + h) * VBW
                                nc.tensor.matmul(
                                    ps_y[gi][:, 0:AVN],
                                    ex[:, gi * P : (gi + 1) * P],
                                    v_sb[:, voff : voff + AVN],
                                    start=(j == 0),
                                    stop=(j == i),
                                )
                        # weave previous q-group's projection between AV
                        # matmuls: fills exp-latency and normalize bubbles
                        if G > 0 and j < 4:
                            emit_proj(G - 1, 4 * (G - 1) + h, j)
                    for gi in range(4):
                        rec = r_pool.tile([P, 1], F32)
                        nc.vector.reciprocal(rec[:], ps_y[gi][:, HD : HD + 1])
                        y_n = y_pool.tile([P, P], F32)
                        nc.vector.tensor_scalar_mul(y_n[:], ps_y[gi][:, 0:HD], rec[:])
                        ps_t = ps_y_pool.tile(
                            [P, 2 * AVN], F32, name=f"pstr_{h}_{G}_{gi}", tag="psy"
                        )
                        nc.tensor.transpose(ps_t[:, 0:P], y_n[:], ident[:])
                        nc.vector.tensor_copy(
                            yt_sb[h][G][:, gi * P : (gi + 1) * P], ps_t[:, 0:P]
                        )
            k = 0
            for tt in range(4 * (nqg - 1), 4 * nqg):
                for og in range(nog):
                    emit_proj(nqg - 1, tt, og, pool_pick=k % 3)
                    k += 1

    nc.compile()
    return nc


def _rope_tables(t=T):
    inv_freq = 1.0 / (10000.0 ** (np.arange(0, HD, 2, dtype=np.float64) / HD))
    ts = np.arange(t, dtype=np.float64)
    freqs = np.outer(ts, inv_freq)  # [t, 64]
    emb = np.concatenate([freqs, freqs], axis=-1)  # [t, 128]
    cos = np.cos(emb).astype(np.float32)
    sin = np.sin(emb).astype(np.float32)
    cosT = np.ascontiguousarray(cos.T)  # [128, t]
    # rotate_half sign lives in the S matrix; sin table is used as-is
    return cosT, np.ascontiguousarray(sin.T)


def _consts(t=T):
    cosT, sinT2 = _rope_tables(t)
    S = np.zeros((P, P), dtype=np.float32)
    for m in range(64):
        S[m, m + 64] = -1.0
    for m in range(64, 128):
        S[m, m - 64] = 1.0
    ST = np.ascontiguousarray(S.T)
    # scoresT layout [tk, tq]: valid iff tk <= tq
    mask = np.where(
        np.arange(P)[:, None] <= np.arange(P)[None, :], 0.0, -1e30
    ).astype(np.float32)
    return cosT, sinT2, ST, mask


def _core_in_map(x_b, w_qkv, w_out, g, t=T):
    cosT, sinT2, ST, mask = _consts(t)
    ntt = t // P
    vones = np.zeros((P, ntt * NH, 4), dtype=np.float32)
    vones[:, :, 0] = 1.0
    d2 = w_qkv.shape[1]
    q_rows = w_qkv[512 * g : 512 * (g + 1)]
    k_rows = w_qkv[d2 + 512 * g : d2 + 512 * (g + 1)]
    v_rows = w_qkv[2 * d2 + 512 * g : 2 * d2 + 512 * (g + 1)]
    return {
        "xt": np.ascontiguousarray(x_b.T),
        "wqkt": np.ascontiguousarray(np.concatenate([q_rows, k_rows], axis=0).T),
        "wvt": np.ascontiguousarray(v_rows.T),
        "wot": np.ascontiguousarray(w_out[:, 512 * g : 512 * (g + 1)].T),
        "cost": cosT,
        "sint": sinT2,
        "st": ST,
        "mask": mask,
        "vones": vones,
        "vtail": np.zeros((P, VPAD), dtype=np.float32),
    }


def kernel(x, w_qkv, w_out):
    global LAST_RESULTS
    x = np.ascontiguousarray(np.asarray(x, dtype=np.float32))
    w_qkv = np.ascontiguousarray(np.asarray(w_qkv, dtype=np.float32))
    w_out = np.ascontiguousarray(np.asarray(w_out, dtype=np.float32))

    if "nc" not in _CACHE:
        _CACHE["nc"] = _build_program()
    nc = _CACHE["nc"]

    B = x.shape[0]
    in_maps = [_core_in_map(x[c // 4], w_qkv, w_out, c % 4) for c in range(8)]
    res = bass_utils.run_bass_kernel_spmd(nc, in_maps, core_ids=list(range(8)))
    LAST_RESULTS = res
    out = np.zeros((B, T, DIM), dtype=np.float32)
    for c in range(8):
        out[c // 4] += res.results[c]["out"]
    return out


if __name__ == "__main__":
    t0 = time.time()
    _CACHE["nc"] = _build_program()
    print(f"program built+compiled in {time.time()-t0:.1f}s")

